# revision 1
# baseline (speedup 1.0000x reference)
"""Trainium2 Bass kernel for single-head attention (no V projection).

Reference computation (per batch b):
    q = x @ Wq ; k = x @ Wk
    scores = q @ k.T / sqrt(64)
    out = softmax(scores, axis=-1) @ x

Shapes: x [4, 2048, 1024], Wq/Wk [1024, 1024] -> out [4, 2048, 1024] fp32.

Sharding: 8 cores, core c handles batch b=c//2, query-row half h=c%2.
Each core receives its batch's x rolled so its 1024 query rows come
first (attention is permutation-invariant over keys), plus the same x
pre-transposed on the host (xt) - the PE contracts over the partition
dim, so the q/k projections need x with the model dim on partitions,
and trn2 has no fp32 DMA-transpose while PE-transpose costs ~300ns per
128x128 tile. Wq is pre-scaled by 1/8 on the host so the softmax scale
is free. No collectives: each core redundantly computes k for its
whole batch (a pair-wise AllGather of the k halves was measured ~90us
end-to-end on this stack - far more than the 27us of PE it saves).

On-chip dataflow (all matmuls contract over the partition dim):
    qT  [e, s]  = Wq.T @ x.T   (lhsT=Wq tile, rhs=xt)
    kT  [e, t]  = Wk.T @ x.T
    scoresT [t, s] = kT.T-chunks @ qT   (lhsT=kT, rhs=qT)
    expT = Exp(scoresT)        (ScalarE eviction from PSUM)
    sumexp [s, 2] = expT.T @ ones      (N=2 matmuls; fp32r needs N>=2)
    out [s, d] = expT.T @ x            (lhsT=expT, rhs=x natural)
    out scaled by 1/sumexp on the DVE during PSUM->SBUF eviction.

A burst of tiny warmup matmuls on the ones tile runs as soon as its
64KB lands (~9us) so the PE's HAM clock-gate reaches 2.4 GHz before
the first real projection matmuls issue (otherwise the first ~12us of
projections run at 1.2 GHz).

Matmul operands live in SBUF as float32r (fp32 bits; the PE truncates
to FP22 on read - 1 cycle/row at free-dim>=256 vs 4 for fp32; measured
end-to-end rel err ~6e-4). The BIR verifier requires every producer of
an fp32r matmul operand to write fp32r-typed data, so DMA'd tiles use
fp32r DRAM params and PSUM evictions write to fp32r tiles.

Softmax skips the max-subtraction: scores have std ~4 and |max| < ~25,
so exp stays comfortably inside fp32 range and the result is
mathematically identical to jax.nn.softmax.
"""

from contextlib import ExitStack

import numpy as np

import concourse.bacc as bacc
import concourse.tile as tile
from concourse import mybir
from concourse.bass_utils import run_bass_kernel_spmd

F32 = mybir.dt.float32
F32R = mybir.dt.float32r
AFT = mybir.ActivationFunctionType

P = 128      # partitions
S = 2048     # keys (t) per batch
SQ = 1024    # query rows per core
D = 1024     # model dim
NT = S // P  # 16 t-chunks
ND = D // P  # 8 d/e-chunks
SB = 512     # query-block width in phase C
NSB = SQ // SB
N_WARMUP = 32   # N=512 matmuls on uninitialized SBUF (no DMA dependency,
                # results never read) sized to end right as the first
                # projection inputs land (~17us): the HAM clock-gate
                # reaches 2.4GHz by ~11us and must not see a >3.4us idle
                # window before the real matmuls start. 80 of these
                # overshot (+10us start delay); N=2 warmups never warm
                # the gate (the array is busy ~2 cycles per dispatch).

B_FULL, S_FULL, D_FULL = 4, 2048, 1024
N_CORES = 8

_NC_CACHE = None
LAST_RESULT = None  # BassKernelResults of the most recent kernel() call
TRACE = False      # set by test.py to capture an NTFF profile
TRACE_DIR = None


def _r(ap):
    return ap.bitcast(F32R)


def _build_nc():
    global _NC_CACHE
    if _NC_CACHE is not None:
        return _NC_CACHE

    nc = bacc.Bacc("TRN2")
    x = nc.declare_dram_parameter("x", [S, D], F32R, isOutput=False)
    xt = nc.declare_dram_parameter("xt", [D, S], F32R, isOutput=False)
    wq = nc.declare_dram_parameter("wq", [D, D], F32R, isOutput=False)
    wk = nc.declare_dram_parameter("wk", [D, D], F32R, isOutput=False)
    # wk's e=0 column-slices pre-gathered on the host into SBUF layout
    # (wk0[p, dd*128+j] = wk[dd*128+p, j]): the first kT matmul group
    # needs them before anything else, and as strided slices of wk they
    # stream at 512-byte packets; contiguous they move in one 4KB-row DMA.
    wk0 = nc.declare_dram_parameter("wk0", [P, ND * P], F32R, isOutput=False)
    ones = nc.declare_dram_parameter("ones", [P, 2], F32R, isOutput=False)
    out = nc.declare_dram_parameter("out", [SQ, D], F32, isOutput=True)

    with tile.TileContext(nc) as tc, ExitStack() as ctx:
        singles = ctx.enter_context(tc.tile_pool(name="singles", bufs=1))
        ot = singles.tile([P, 2], F32R)
        nc.sync.dma_start(out=ot[:], in_=ones[:])

        persist = ctx.enter_context(tc.tile_pool(name="persist", bufs=1))
        # e-chunk e lives at [:, e*SQ : (e+1)*SQ] (free axis = s)
        qT = persist.tile([P, ND * SQ], F32R, tag="qT")
        # e-chunk e lives at [:, e*S : (e+1)*S] (free axis = t)
        kT = persist.tile([P, ND * S], F32R, tag="kT")

        # ---------- phase B: load xt/w, project k then q ----------
        # kT goes first: its 55us of matmuls cover the 12MB xt+wk input
        # stream, so only the ~17us head (startup + first 2.5MB) is
        # DMA-bound. SBUF is too tight for two full 4MB weight buffers
        # alongside xT/qT/kT, so wq prefetches its first 3 column-blocks
        # (e<3, 1.5MB) into wB during the kT matmuls; the remaining
        # columns reuse wk's slot afterwards, hidden behind the e<3 qT
        # groups.
        NQB = 2  # wq e-chunks prefetched into wB (shrunk to fit wk0)
        with tc.tile_pool(name="xT", bufs=1) as xT_pool, \
             tc.tile_pool(name="wA", bufs=1) as wA_pool, \
             tc.tile_pool(name="wB", bufs=1) as wB_pool, \
             tc.tile_pool(name="wk0p", bufs=1) as wk0_pool, \
             tc.tile_pool(name="ps_w", bufs=1, space="PSUM") as ps_w_pool, \
             tc.tile_pool(name="ps_pj", bufs=4, space="PSUM") as ps_pj:
            # d-chunk d at [:, d*S : (d+1)*S] (free axis = s)
            xT = xT_pool.tile([P, ND * S], F32R)
            wt = wA_pool.tile([P, ND * D], F32R, tag="w")

            # PE warmup on uninitialized qT contents (qT's first real
            # write is a DVE eviction much later, so no dependency and no
            # delay); garbage results land in a scratch PSUM bank that is
            # never read.
            ps_w = ps_w_pool.tile([P, 512], F32)
            for i in range(N_WARMUP):
                nc.tensor.matmul(ps_w[:], qT[:, 0:P], qT[:, SQ:SQ + 512],
                                 start=(i == 0), stop=(i == N_WARMUP - 1))

            # DMA order drives arrival order. The first kT psum group
            # (st=0, e=0) needs only wk0 + xt s<512, so stream those
            # ~2.5MB first.
            wk0_t = wk0_pool.tile([P, ND * P], F32R)
            nc.sync.dma_start(out=wk0_t[:], in_=wk0[:])
            for d in range(ND):
                nc.sync.dma_start(
                    out=xT[:, d * S: d * S + 512],
                    in_=xt[d * P:(d + 1) * P, 0:512],
                )
            for dd in range(ND):
                nc.sync.dma_start(
                    out=wt[:, dd * D + P:(dd + 1) * D],
                    in_=wk[dd * P:(dd + 1) * P, P:],
                )
            for st in range(1, 4):
                for d in range(ND):
                    nc.sync.dma_start(
                        out=xT[:, d * S + st * 512: d * S + st * 512 + 512],
                        in_=xt[d * P:(d + 1) * P, st * 512: st * 512 + 512],
                    )
            # wq columns e<NQB into wB (streams during kT matmuls);
            # wB[:, dd*NQB*P ...] holds wq[dd-chunk, 0:NQB*128]
            wqB = wB_pool.tile([P, NQB * D], F32R)
            for dd in range(ND):
                nc.sync.dma_start(
                    out=wqB[:, dd * NQB * P:(dd + 1) * NQB * P],
                    in_=wq[dd * P:(dd + 1) * P, 0:NQB * P],
                )

            # kT projection (full t=2048); 512-wide stripes only -
            # narrower ones are LDWEIGHTS-bound (190ns load > 107ns mm)
            for st in range(S // 512):
                for e in range(ND):
                    ps = ps_pj.tile([P, 512], F32)
                    for dd in range(ND):
                        if e == 0:
                            lhs = wk0_t[:, dd * P:(dd + 1) * P]
                        else:
                            lhs = wt[:, dd * D + e * P: dd * D + (e + 1) * P]
                        nc.tensor.matmul(
                            ps[:],
                            lhs,
                            xT[:, dd * S + st * 512: dd * S + st * 512 + 512],
                            start=(dd == 0), stop=(dd == ND - 1),
                        )
                    nc.vector.tensor_copy(
                        kT[:, e * S + st * 512: e * S + st * 512 + 512],
                        _r(ps[:]),
                    )

            # wq columns e>=NQB reuse wk's slot once kT releases it
            NQR = ND - NQB
            wqR = wA_pool.tile([P, NQR * D], F32R, tag="w")
            for dd in range(ND):
                nc.sync.dma_start(
                    out=wqR[:, dd * NQR * P:(dd + 1) * NQR * P],
                    in_=wq[dd * P:(dd + 1) * P, NQB * P:],
                )

            # qT projection; e ascending so the e<NQB groups (lhsT in
            # wB) cover the wqR refill
            for e in range(ND):
                for sh in range(SQ // 512):
                    ps = ps_pj.tile([P, 512], F32)
                    for dd in range(ND):
                        if e < NQB:
                            lhs = wqB[:, dd * NQB * P + e * P:
                                      dd * NQB * P + (e + 1) * P]
                        else:
                            lhs = wqR[:, dd * NQR * P + (e - NQB) * P:
                                      dd * NQR * P + (e - NQB + 1) * P]
                        nc.tensor.matmul(
                            ps[:],
                            lhs,
                            xT[:, dd * S + sh * 512: dd * S + sh * 512 + 512],
                            start=(dd == 0), stop=(dd == ND - 1),
                        )
                    nc.vector.tensor_copy(
                        qT[:, e * SQ + sh * 512: e * SQ + sh * 512 + 512],
                        _r(ps[:]),
                    )

        # ---------- phase C: scores -> softmax -> attn @ x ----------
        with tc.tile_pool(name="xc", bufs=1) as xc_pool, \
             tc.tile_pool(name="exp", bufs=1) as exp_pool, \
             tc.tile_pool(name="outp", bufs=4) as out_pool, \
             tc.tile_pool(name="recip", bufs=4) as recip_pool, \
             tc.tile_pool(name="partial", bufs=2) as partial_pool, \
             tc.tile_pool(name="ps_sc", bufs=4, space="PSUM") as ps_sc, \
             tc.tile_pool(name="ps_av", bufs=3, space="PSUM") as ps_av, \
             tc.tile_pool(name="ps_sum", bufs=1, space="PSUM") as ps_sum:
            # x natural: t-chunk t at [:, t*D : (t+1)*D]
            xc = xc_pool.tile([P, NT * D], F32R)
            for t in range(NT):
                nc.sync.dma_start(
                    out=xc[:, t * D:(t + 1) * D], in_=x[t * P:(t + 1) * P, :]
                )

            for blk in range(NSB):
                # t-chunk t at [:, t*SB : (t+1)*SB] (free axis = s within blk)
                expT = exp_pool.tile([P, NT * SB], F32R, tag="expT")
                # Softmax denominator: the 16-chunk accumulation runs on
                # the (otherwise idle) DVE as a chain of adds interleaved
                # with the scores loop; the PE then only does one N=2
                # partition-reduce matmul per s-chunk instead of 16
                # LDWEIGHTS-bound ones each (~115ns apiece, ~18us total).
                partial = partial_pool.tile([P, SB], F32R, tag="partial")
                for t in range(NT):
                    ps = ps_sc.tile([P, SB], F32)
                    for e in range(ND):
                        nc.tensor.matmul(
                            ps[:],
                            kT[:, e * S + t * P: e * S + (t + 1) * P],
                            qT[:, e * SQ + blk * SB: e * SQ + (blk + 1) * SB],
                            start=(e == 0), stop=(e == ND - 1),
                        )
                    nc.scalar.activation(expT[:, t * SB:(t + 1) * SB], ps[:], AFT.Exp)
                    if t == 1:
                        nc.vector.tensor_add(
                            partial[:], expT[:, 0:SB], expT[:, SB:2 * SB])
                    elif t >= 2:
                        nc.vector.tensor_add(
                            partial[:], partial[:],
                            expT[:, t * SB:(t + 1) * SB])

                for ss in range(SB // P):
                    pss = ps_sum.tile([P, 2], F32)
                    nc.tensor.matmul(
                        pss[:], partial[:, ss * P:(ss + 1) * P], ot[:],
                        start=True, stop=True,
                    )
                    rec = recip_pool.tile([P, 1], F32, tag="rec")
                    nc.vector.reciprocal(rec[:], pss[:, 0:1])

                    for dh in range(2):
                        psa = ps_av.tile([P, 512], F32)
                        for t in range(NT):
                            nc.tensor.matmul(
                                psa[:],
                                expT[:, t * SB + ss * P: t * SB + (ss + 1) * P],
                                xc[:, t * D + dh * 512: t * D + dh * 512 + 512],
                                start=(t == 0), stop=(t == NT - 1),
                            )
                        ob = out_pool.tile([P, 512], F32, tag="ob")
                        nc.vector.tensor_scalar_mul(ob[:], psa[:], rec[:, 0:1])
                        row0 = blk * SB + ss * P
                        nc.sync.dma_start(
                            out=out[row0:row0 + P, dh * 512:dh * 512 + 512],
                            in_=ob[:],
                        )

    nc.finalize()
    _NC_CACHE = nc
    return nc


def kernel(inputs, Wq, Wk):
    global LAST_RESULT
    x = np.asarray(inputs, dtype=np.float32)
    assert x.shape == (B_FULL, S_FULL, D_FULL)
    wq = np.ascontiguousarray(np.asarray(Wq, dtype=np.float32) * np.float32(0.125))
    wk = np.ascontiguousarray(np.asarray(Wk, dtype=np.float32))
    wk0 = np.ascontiguousarray(
        wk.reshape(ND, P, D)[:, :, 0:P].transpose(1, 0, 2).reshape(P, ND * P))
    ones = np.ones((P, 2), dtype=np.float32)

    nc = _build_nc()

    in_maps = []
    for c in range(N_CORES):
        b, h = c // 2, c % 2
        xb = x[b]
        if h:
            xb = np.concatenate([xb[SQ:], xb[:SQ]], axis=0)
        in_maps.append({
            "x": np.ascontiguousarray(xb),
            "xt": np.ascontiguousarray(xb.T),
            "wq": wq,
            "wk": wk,
            "wk0": wk0,
            "ones": ones,
        })

    kwargs = {}
    if TRACE:
        kwargs = {"trace": True, "tmpdir": TRACE_DIR}
    res = run_bass_kernel_spmd(nc, in_maps, list(range(N_CORES)), **kwargs)
    LAST_RESULT = res

    full = np.empty((B_FULL, S_FULL, D_FULL), dtype=np.float32)
    for c in range(N_CORES):
        b, h = c // 2, c % 2
        full[b, h * SQ:(h + 1) * SQ, :] = res.results[c]["out"]
    return full



# revision 2
# speedup vs baseline: 1.3385x; 1.3385x over previous
"""Trainium2 Bass kernel for single-head attention (no V projection).

Reference computation (per batch b):
    q = x @ Wq ; k = x @ Wk
    scores = q @ k.T / sqrt(64)
    out = softmax(scores, axis=-1) @ x

Key algebraic rewrite: scores = (x Wq)(x Wk)^T / 8 = x A x^T with
A = Wq Wk^T / 8 precomputed on the host. Each core then projects only
its OWN query rows (y = x_q @ A) and uses x^T (already resident in
SBUF for the projection) directly as the scores lhsT — the entire k
projection (2048x1024x1024 per core, ~74us of PE time including the
cross-core redundancy) disappears. Per-core PE work drops from
15.0 GF to 10.75 GF with no collectives and identical statistics
(A ~ N(0,1/D) like Wq, y ~ N(0,1) like q).

Shapes: x [4, 2048, 1024], Wq/Wk [1024, 1024] -> out [4, 2048, 1024] fp32.

Sharding: 8 cores, core c handles batch b=c//2, query-row half h=c%2.
Each core receives its batch's x rolled so its 1024 query rows come
first (attention is permutation-invariant over keys), plus the same x
pre-transposed on the host (xt) — the PE contracts over the partition
dim, so the y projection and scores need x with the model dim on
partitions, and trn2 has no fp32 DMA-transpose.

On-chip dataflow (all matmuls contract over the partition dim):
    yT  [e, s]  = A.T @ x_q.T     (lhsT=A tile, rhs=xT)
    scoresT [t, s] = xT-chunks.T @ yT   (lhsT=xT, rhs=yT)
    expT = Exp(scoresT)           (ScalarE eviction from PSUM)
    sumexp [s, 2] = expT.T @ ones (N=2 matmuls; fp32r needs N>=2)
    out [s, d] = expT.T @ x       (lhsT=expT, rhs=x natural)
    out scaled by 1/sumexp on the DVE during PSUM->SBUF eviction.

A is pre-arranged on the host into the e-major SBUF layout
(wa[p, e*D + dd*P + j] = A[dd*P+p, e*P+j]) so it streams as ND
contiguous 512KB DMAs whose arrival order matches the yT loop's
consumption order (e ascending).

A burst of tiny warmup matmuls on uninitialized SBUF runs immediately
(no DMA dependency; results land in a never-read PSUM bank) so the
PE's HAM clock-gate ramps to 2.4 GHz before the first real projection
matmuls issue (~12 matmuls run at 1.2-2.4GHz ramp otherwise).

Matmul operands live in SBUF as float32r (fp32 bits; the PE truncates
to FP22 on read — 1 cycle/row at free-dim>=256 vs 4 for fp32; measured
end-to-end rel err ~6e-4). The BIR verifier requires every producer of
an fp32r matmul operand to write fp32r-typed data, so DMA'd tiles use
fp32r DRAM params and PSUM evictions write to fp32r tiles.

Softmax skips the max-subtraction: scores have std ~4 and |max| < ~25,
so exp stays comfortably inside fp32 range and the result is
mathematically identical to jax.nn.softmax.
"""

from contextlib import ExitStack

import numpy as np

import concourse.bacc as bacc
import concourse.tile as tile
from concourse import mybir
from concourse.bass_utils import run_bass_kernel_spmd

F32 = mybir.dt.float32
F32R = mybir.dt.float32r
AFT = mybir.ActivationFunctionType

P = 128      # partitions
S = 2048     # keys (t) per batch
SQ = 1024    # query rows per core
D = 1024     # model dim
NT = S // P  # 16 t-chunks
ND = D // P  # 8 d/e-chunks
SB = 512     # query-block width in phase C
NSB = SQ // SB
N_WARMUP = 24   # N=512 matmuls on uninitialized SBUF (no DMA dependency,
                # results never read) sized to end right as the first
                # projection inputs land (~8us): ~12 run during the
                # clock ramp (427-795ns each), the rest at full speed
                # (227ns), ending ~7.5us in.

B_FULL, S_FULL, D_FULL = 4, 2048, 1024
N_CORES = 8

_NC_CACHE = None
LAST_RESULT = None  # BassKernelResults of the most recent kernel() call
TRACE = False      # set by test.py to capture an NTFF profile
TRACE_DIR = None


def _r(ap):
    return ap.bitcast(F32R)


def _build_nc():
    global _NC_CACHE
    if _NC_CACHE is not None:
        return _NC_CACHE

    nc = bacc.Bacc("TRN2")
    x = nc.declare_dram_parameter("x", [S, D], F32R, isOutput=False)
    xt = nc.declare_dram_parameter("xt", [D, S], F32R, isOutput=False)
    # A = Wq @ Wk.T / 8, pre-arranged e-major (see module docstring)
    wa = nc.declare_dram_parameter("wa", [P, ND * D], F32R, isOutput=False)
    ones = nc.declare_dram_parameter("ones", [P, 2], F32R, isOutput=False)
    out = nc.declare_dram_parameter("out", [SQ, D], F32, isOutput=True)

    with tile.TileContext(nc) as tc, ExitStack() as ctx:
        singles = ctx.enter_context(tc.tile_pool(name="singles", bufs=1))
        ot = singles.tile([P, 2], F32R)
        nc.sync.dma_start(out=ot[:], in_=ones[:])

        persist = ctx.enter_context(tc.tile_pool(name="persist", bufs=1))
        # d-chunk d at [:, d*S : (d+1)*S] (free axis = s over all 2048
        # keys); doubles as the scores lhsT in phase C.
        xT = persist.tile([P, ND * S], F32R, tag="xT")
        # e-chunk e at [:, e*SQ : (e+1)*SQ] (free axis = s query)
        yT = persist.tile([P, ND * SQ], F32R, tag="yT")
        # x natural: t-chunk t at [:, t*D : (t+1)*D]
        xc = persist.tile([P, NT * D], F32R, tag="xc")

        # ---------- phase B: load xt/A, project y ----------
        with tc.tile_pool(name="wA", bufs=1) as wA_pool, \
             tc.tile_pool(name="ps_w", bufs=1, space="PSUM") as ps_w_pool, \
             tc.tile_pool(name="ps_pj", bufs=4, space="PSUM") as ps_pj:
            wt = wA_pool.tile([P, ND * D], F32R, tag="w")

            # PE warmup on uninitialized yT contents (yT's first real
            # write is a DVE eviction later, so no dependency and no
            # delay); garbage results land in a scratch PSUM bank that
            # is never read.
            ps_w = ps_w_pool.tile([P, 512], F32)
            for i in range(N_WARMUP):
                nc.tensor.matmul(ps_w[:], yT[:, 0:P], yT[:, SQ:SQ + 512],
                                 start=(i == 0), stop=(i == N_WARMUP - 1))

            # DMA order drives arrival order. Group (e=0, sh=0) needs
            # xT s<512 + A's e=0 block (~2.5MB); stream those first,
            # then interleave so each yT group's inputs land just ahead
            # of its issue, then the phase-C key stripes, then xc.
            for d in range(ND):
                nc.sync.dma_start(
                    out=xT[:, d * S: d * S + 512],
                    in_=xt[d * P:(d + 1) * P, 0:512],
                )
            nc.sync.dma_start(out=wt[:, 0:D], in_=wa[:, 0:D])
            for d in range(ND):
                nc.sync.dma_start(
                    out=xT[:, d * S + 512: d * S + 1024],
                    in_=xt[d * P:(d + 1) * P, 512:1024],
                )
            for e in range(1, ND):
                nc.sync.dma_start(
                    out=wt[:, e * D:(e + 1) * D], in_=wa[:, e * D:(e + 1) * D]
                )
            for st in range(2, 4):
                for d in range(ND):
                    nc.sync.dma_start(
                        out=xT[:, d * S + st * 512: d * S + st * 512 + 512],
                        in_=xt[d * P:(d + 1) * P, st * 512: st * 512 + 512],
                    )
            for t in range(NT):
                nc.sync.dma_start(
                    out=xc[:, t * D:(t + 1) * D], in_=x[t * P:(t + 1) * P, :]
                )

            # yT projection; e ascending matches A's DMA arrival order,
            # 512-wide stripes only — narrower ones are LDWEIGHTS-bound
            for e in range(ND):
                for sh in range(SQ // 512):
                    ps = ps_pj.tile([P, 512], F32)
                    for dd in range(ND):
                        nc.tensor.matmul(
                            ps[:],
                            wt[:, e * D + dd * P: e * D + (dd + 1) * P],
                            xT[:, dd * S + sh * 512: dd * S + sh * 512 + 512],
                            start=(dd == 0), stop=(dd == ND - 1),
                        )
                    nc.vector.tensor_copy(
                        yT[:, e * SQ + sh * 512: e * SQ + sh * 512 + 512],
                        _r(ps[:]),
                    )

        # ---------- phase C: scores -> softmax -> attn @ x ----------
        with tc.tile_pool(name="exp", bufs=1) as exp_pool, \
             tc.tile_pool(name="outp", bufs=4) as out_pool, \
             tc.tile_pool(name="recip", bufs=4) as recip_pool, \
             tc.tile_pool(name="partial", bufs=2) as partial_pool, \
             tc.tile_pool(name="ps_sc", bufs=4, space="PSUM") as ps_sc, \
             tc.tile_pool(name="ps_av", bufs=3, space="PSUM") as ps_av, \
             tc.tile_pool(name="ps_sum", bufs=1, space="PSUM") as ps_sum:
            for blk in range(NSB):
                # t-chunk t at [:, t*SB : (t+1)*SB] (free axis = s within blk)
                expT = exp_pool.tile([P, NT * SB], F32R, tag="expT")
                # Softmax denominator: the 16-chunk accumulation runs on
                # the (otherwise idle) DVE as a chain of adds interleaved
                # with the scores loop; the PE then only does one N=2
                # partition-reduce matmul per s-chunk instead of 16
                # LDWEIGHTS-bound ones each (~115ns apiece, ~18us total).
                partial = partial_pool.tile([P, SB], F32R, tag="partial")
                for t in range(NT):
                    ps = ps_sc.tile([P, SB], F32)
                    for dd in range(ND):
                        nc.tensor.matmul(
                            ps[:],
                            xT[:, dd * S + t * P: dd * S + (t + 1) * P],
                            yT[:, dd * SQ + blk * SB: dd * SQ + (blk + 1) * SB],
                            start=(dd == 0), stop=(dd == ND - 1),
                        )
                    nc.scalar.activation(expT[:, t * SB:(t + 1) * SB], ps[:], AFT.Exp)
                    if t == 1:
                        nc.vector.tensor_add(
                            partial[:], expT[:, 0:SB], expT[:, SB:2 * SB])
                    elif t >= 2:
                        nc.vector.tensor_add(
                            partial[:], partial[:],
                            expT[:, t * SB:(t + 1) * SB])

                for ss in range(SB // P):
                    pss = ps_sum.tile([P, 2], F32)
                    nc.tensor.matmul(
                        pss[:], partial[:, ss * P:(ss + 1) * P], ot[:],
                        start=True, stop=True,
                    )
                    rec = recip_pool.tile([P, 1], F32, tag="rec")
                    nc.vector.reciprocal(rec[:], pss[:, 0:1])

                    for dh in range(2):
                        psa = ps_av.tile([P, 512], F32)
                        for t in range(NT):
                            nc.tensor.matmul(
                                psa[:],
                                expT[:, t * SB + ss * P: t * SB + (ss + 1) * P],
                                xc[:, t * D + dh * 512: t * D + dh * 512 + 512],
                                start=(t == 0), stop=(t == NT - 1),
                            )
                        ob = out_pool.tile([P, 512], F32, tag="ob")
                        nc.vector.tensor_scalar_mul(ob[:], psa[:], rec[:, 0:1])
                        row0 = blk * SB + ss * P
                        nc.sync.dma_start(
                            out=out[row0:row0 + P, dh * 512:dh * 512 + 512],
                            in_=ob[:],
                        )

    nc.finalize()
    _NC_CACHE = nc
    return nc


def kernel(inputs, Wq, Wk):
    global LAST_RESULT
    x = np.asarray(inputs, dtype=np.float32)
    assert x.shape == (B_FULL, S_FULL, D_FULL)
    A = (np.asarray(Wq, dtype=np.float32) @ np.asarray(Wk, dtype=np.float32).T
         ) * np.float32(0.125)
    # wa[p, e*D + dd*P + j] = A[dd*P + p, e*P + j]
    wa = np.ascontiguousarray(
        A.reshape(ND, P, ND, P).transpose(1, 2, 0, 3).reshape(P, ND * D))
    ones = np.ones((P, 2), dtype=np.float32)

    nc = _build_nc()

    in_maps = []
    for c in range(N_CORES):
        b, h = c // 2, c % 2
        xb = x[b]
        if h:
            xb = np.concatenate([xb[SQ:], xb[:SQ]], axis=0)
        in_maps.append({
            "x": np.ascontiguousarray(xb),
            "xt": np.ascontiguousarray(xb.T),
            "wa": wa,
            "ones": ones,
        })

    kwargs = {}
    if TRACE:
        kwargs = {"trace": True, "tmpdir": TRACE_DIR}
    res = run_bass_kernel_spmd(nc, in_maps, list(range(N_CORES)), **kwargs)
    LAST_RESULT = res

    full = np.empty((B_FULL, S_FULL, D_FULL), dtype=np.float32)
    for c in range(N_CORES):
        b, h = c // 2, c % 2
        full[b, h * SQ:(h + 1) * SQ, :] = res.results[c]["out"]
    return full
